# revision 9
# baseline (speedup 1.0000x reference)
"""Masked causal multi-head attention on 8 TRN2 NeuronCores.

Problem (hardcoded shapes): B=4, S=2048, D_MODEL=1024, HEADS=16,
KEY_SIZE=SIZE_PER_HEAD=64, OUT_DIM=1024, fp32 I/O.

Sharding: pure data/tensor parallel — core c handles batch b=c//2 and
head-group hg=c%2 (8 heads). Each core's output shard [2048, 512] is
independent, so there are no collectives; the host assembles shards.

Per-core pipeline (all matmul inputs bf16; PSUM accumulation fp32):
  - head-PAIR processing: even head on PE rows 0-63, odd head on rows
    64-127 (concurrent row groups) for the score matmuls; both heads'
    scores land in ONE bf16 PSUM bank [128 keys, 2, 512 q], so a
    single ScalarE activation (FD=1024) does exp for both heads —
    ScalarE instruction count (the attention-phase critical path) is
    half of the per-head variant.
  - exp((S)/8 + key_bias) with NO max-subtraction (masked lanes
    underflow to exactly 0, matching the reference's -1e10 additive
    masks); causal masking of the diagonal block is a 0/1 multiply on
    P AFTER exp (both heads in one VectorE op), off the critical path.
  - PE stream is software-pipelined: scores(kt+1) are emitted BEFORE
    PV(kt) so the PE never sits blocked on the exp of tile kt while
    score work for kt+1 is ready.
  - projections run weight-stationary (t-outer over 2 s-chunks with 2
    live PSUM accumulators) so each LDWEIGHTS is reused; pair p+1's
    qw/kw projections are emitted early inside pair p's attention so
    the Tile scheduler can fill PE gaps.
  - vw [s, dh] (bf16) carries a fused ones-column per head (65th col)
    so the PV matmul also emits softmax denominators.
  host: divide by sums, multiply q_mask, transpose, assemble.
"""

import os
import sys

import numpy as np

for _p in ("/opt/trn_rl_repo",):
    if _p not in sys.path and os.path.isdir(_p):
        sys.path.insert(0, _p)

import ml_dtypes

import concourse.bass as bass
import concourse.mybir as mybir
import concourse.tile as tile
from concourse import bacc
from concourse.bass_utils import run_bass_kernel_spmd

B = 4
S = 2048
D = 1024
HEADS_PER_CORE = 8
DH = 64
HG_COLS = HEADS_PER_CORE * DH  # 512 output cols per core
NKT = S // 128  # 16 k-tiles
NEG = -1.0e9
QC = 512  # q-chunk width (one PSUM bank)

F32 = mybir.dt.float32
BF16 = mybir.dt.bfloat16
NP_BF16 = ml_dtypes.bfloat16

LAST_RESULT = None  # stashed BassKernelResults for test harness inspection
_NC_CACHE = None


def _ensure_ntff_hook():
    """The agent image's antenv lacks axon_hooks; synthesize it so
    run_bass_kernel_spmd(trace=True) can reach the NTFF profiler."""
    try:
        from antenv.axon_hooks import get_axon_ntff_profile_hook  # noqa: F401

        return
    except ImportError:
        pass
    import types

    try:
        import antenv
        from trn_agent_boot.trn_boot import _ntff_profile_via_ctypes
    except ImportError:
        return
    mod = types.ModuleType("antenv.axon_hooks")
    _hook = [None]
    try:
        _hook[0] = _ntff_profile_via_ctypes("/opt/axon/libaxon_pjrt.so")
    except OSError:
        pass
    mod.set_axon_ntff_profile_hook = lambda h: _hook.__setitem__(0, h)
    mod.get_axon_ntff_profile_hook = lambda: _hook[0]
    sys.modules["antenv.axon_hooks"] = mod
    antenv.axon_hooks = mod


def _build_nc() -> bass.Bass:
    nc = bacc.Bacc()

    xqT = nc.declare_dram_parameter("xqT", [D, S], BF16, isOutput=False)[:]
    xkT = nc.declare_dram_parameter("xkT", [D, S], BF16, isOutput=False)[:]
    xvT = nc.declare_dram_parameter("xvT", [D, S], BF16, isOutput=False)[:]
    wq = nc.declare_dram_parameter("wq", [D, HG_COLS], BF16, isOutput=False)[:]
    wk = nc.declare_dram_parameter("wk", [D, HG_COLS], BF16, isOutput=False)[:]
    wv = nc.declare_dram_parameter("wv", [D, HG_COLS], BF16, isOutput=False)[:]
    vbias = nc.declare_dram_parameter("vbias", [128, NKT], F32, isOutput=False)[:]
    ctile = nc.declare_dram_parameter("ctile", [128, 2 * 128], BF16, isOutput=False)[:]
    outT = nc.declare_dram_parameter(
        "outT", [HEADS_PER_CORE * 65, S], F32, isOutput=True
    )[:]

    with tile.TileContext(nc) as tc:
        with (
            tc.tile_pool(name="consts", bufs=1) as consts,
            tc.tile_pool(name="xqk", bufs=1) as xqk,
            tc.tile_pool(name="wpool", bufs=1) as wpool,
            tc.tile_pool(name="qk_sb", bufs=1) as qk_sb,
            tc.tile_pool(name="vw_pool", bufs=1) as vw_pool,
            tc.tile_pool(name="xvpool", bufs=2) as xvpool,
            tc.tile_pool(name="ppool", bufs=6) as ppool,
            tc.tile_pool(name="ostage", bufs=4) as ostage,
            tc.tile_pool(name="proj_ps", bufs=2, space="PSUM") as proj_ps,
            tc.tile_pool(name="att_s", bufs=2, space="PSUM") as spool,
            tc.tile_pool(name="att_oe", bufs=1, space="PSUM") as opool_e,
            tc.tile_pool(name="att_oo", bufs=1, space="PSUM") as opool_o,
        ):
            # PE warmup: dependency-free dummy matmuls run during the DMA
            # lead-in so the HAM clock gate opens (K=8/8) before the first
            # real matmul issues
            junk = consts.tile([128, 512], BF16, tag="junk")
            nc.vector.memset(junk, 0.0)
            for _ in range(12):
                wps = proj_ps.tile([128, 512], F32, tag="pp")
                nc.tensor.matmul(
                    wps, junk[:, 0:128], junk, start=True, stop=True
                )

            # DMA strictly in need-order: the first exp gates on
            # wq/wk[dt=0] (256 KB each) + xq0/xk0 + vbias only, so those
            # come first; everything else streams in behind
            wq_sb = wpool.tile([128, 8, HG_COLS], BF16, tag="wq")
            wk_sb = wpool.tile([128, 8, HG_COLS], BF16, tag="wk")
            wq_r = wq.rearrange("(t p) n -> p t n", p=128)
            wk_r = wk.rearrange("(t p) n -> p t n", p=128)

            def dma_w_slice(dt):
                for w_sb, src in ((wq_sb, wq_r), (wk_sb, wk_r)):
                    nc.sync.dma_start(
                        out=w_sb[:, :, dt * 128 : (dt + 1) * 128],
                        in_=src[:, :, dt * 128 : (dt + 1) * 128],
                    )

            xqT_r = xqT.rearrange("(t p) s -> p t s", p=128)
            xkT_r = xkT.rearrange("(t p) s -> p t s", p=128)
            xvT_r = xvT.rearrange("(t p) s -> p t s", p=128)
            xq_cs, xk_cs, xv_ts = [], [], []

            def dma_x_chunk(sc, with_v=True):
                srcs = [(xqT_r, xq_cs, "xq", xqk), (xkT_r, xk_cs, "xk", xqk)]
                if with_v:
                    srcs.append((xvT_r, xv_ts, "xv", xvpool))
                for src, lst, tg, pool in srcs:
                    t_ = pool.tile([128, 8, 512], BF16, tag=f"{tg}{sc}" if pool is xqk else tg)
                    nc.sync.dma_start(out=t_, in_=src[:, :, sc * 512 : (sc + 1) * 512])
                    lst.append(t_)

            dma_w_slice(0)
            dma_x_chunk(0, with_v=False)
            vbias_sb = consts.tile([128, NKT], F32)
            nc.sync.dma_start(out=vbias_sb, in_=vbias)
            ctile_sb = consts.tile([128, 2, 128], BF16)
            nc.sync.dma_start(out=ctile_sb, in_=ctile.rearrange("p (h q) -> p h q", h=2))
            wv_r = wv.rearrange("(t p) n -> t p n", p=128)
            wv_ts = []
            for t in range(8):
                wt = wpool.tile([128, HG_COLS], BF16, tag=f"wv{t}")
                nc.sync.dma_start(out=wt, in_=wv_r[t])
                wv_ts.append(wt)
            xv0 = xvpool.tile([128, 8, 512], BF16, tag="xv", name="xv0")
            nc.sync.dma_start(out=xv0, in_=xvT_r[:, :, 0:512])
            xv_ts.append(xv0)
            for sc in range(1, S // 512):
                dma_x_chunk(sc)
            for dt in range(1, 4):
                dma_w_slice(dt)

            vw_sb = vw_pool.tile([128, NKT, HEADS_PER_CORE, 65], BF16)
            nc.vector.memset(vw_sb[:, :, :, 64:65], 1.0)

            # projected tensors
            qwT_sb = qk_sb.tile([128, 4, S], BF16)  # [dh%128, dh//128, s]
            kwT_sb = qk_sb.tile([128, 4, S], BF16)

            def vw_proj(sc):
                # project vw for k-tiles 4*sc .. 4*sc+3
                for st2 in range(4):
                    kt = sc * 4 + st2
                    ps = proj_ps.tile([128, HG_COLS], F32, tag="pp")
                    for t in range(8):
                        nc.tensor.matmul(
                            ps,
                            xv_ts[sc][:, t, st2 * 128 : (st2 + 1) * 128],
                            wv_ts[t],
                            start=(t == 0),
                            stop=(t == 7),
                        )
                    nc.vector.tensor_copy(
                        vw_sb[:, kt, :, 0:64],
                        ps.rearrange("p (h d) -> p h d", h=HEADS_PER_CORE),
                    )

            def qk_proj(dt, scs):
                # weight-stationary: t-outer over the given s-chunks with
                # one live PSUM accumulator per chunk, so each LDWEIGHTS
                # (the W tile) is reused len(scs) times
                for w_sb, x_cs, dst in (
                    (wq_sb, xq_cs, qwT_sb),
                    (wk_sb, xk_cs, kwT_sb),
                ):
                    pss = [
                        proj_ps.tile([128, 512], F32, tag="pp", name=f"pp{i}")
                        for i in range(len(scs))
                    ]
                    for t in range(8):
                        for ps, sc in zip(pss, scs):
                            nc.tensor.matmul(
                                ps,
                                w_sb[:, t, dt * 128 : (dt + 1) * 128],
                                x_cs[sc][:, t, :],
                                start=(t == 0),
                                stop=(t == 7),
                            )
                    for ps, sc in zip(pss, scs):
                        nc.vector.tensor_copy(
                            dst[:, dt, sc * 512 : (sc + 1) * 512], ps
                        )

            def attention(pair, qc):
                dt = pair
                h_e, h_o = 2 * pair, 2 * pair + 1
                q0 = qc * QC
                out_e = opool_e.tile([65, QC], F32, tag="oute")
                out_o = opool_o.tile([65, QC], F32, tag="outo")
                last_kt = (q0 + QC) // 128 - 1
                pend_pv = None
                for kt in range(last_kt + 1):
                    q_off = max(0, 128 * kt - q0)
                    # both heads' scores into one 2-bank PSUM tile; the
                    # two matmuls use disjoint PE row groups (0 / 64) ->
                    # concurrent
                    s_ps = spool.tile([128, 2 * QC], F32, tag="s")
                    for hi, poff in ((0, 0), (1, 64)):
                        nc.tensor.matmul(
                            s_ps[:, hi * QC + q_off : (hi + 1) * QC],
                            kwT_sb[
                                poff : poff + 64, dt, kt * 128 : (kt + 1) * 128
                            ],
                            qwT_sb[poff : poff + 64, dt, q0 + q_off : q0 + QC],
                            start=True,
                            stop=True,
                        )
                    # ONE exp for both heads (FD up to 1024; flat 2D AP in
                    # the common q_off=0 case)
                    p_t = ppool.tile([128, 2 * QC], BF16, tag="p")
                    if q_off == 0:
                        act_dst, act_src = p_t, s_ps
                    else:
                        act_dst = p_t.rearrange("p (h q) -> p h q", h=2)[:, :, q_off:QC]
                        act_src = s_ps.rearrange("p (h q) -> p h q", h=2)[:, :, q_off:QC]
                    nc.scalar.activation(
                        act_dst,
                        act_src,
                        mybir.ActivationFunctionType.Exp,
                        bias=vbias_sb[:, kt : kt + 1],
                        scale=0.125,
                    )
                    if 128 * kt >= q0:
                        # causal mask of the diagonal block as 0/1
                        # multiply AFTER exp, both heads in one op
                        p_d = p_t.rearrange("p (h q) -> p h q", h=2)[
                            :, :, q_off : q_off + 128
                        ]
                        nc.vector.tensor_mul(p_d, p_d, ctile_sb)
                    # software pipelining: PV(kt-1) is emitted AFTER
                    # scores(kt), so the PE has ready score work queued
                    # while exp(kt-1) finishes
                    if pend_pv is not None:
                        pend_pv()
                    def _pv(kt=kt, p_t=p_t, q_off=q_off):
                        for h, out_ps, hi in ((h_e, out_e, 0), (h_o, out_o, 1)):
                            nc.tensor.matmul(
                                out_ps[:, q_off:QC],
                                vw_sb[:, kt, h, :],
                                p_t[:, hi * QC + q_off : (hi + 1) * QC],
                                start=(kt == 0),
                                stop=(kt == last_kt),
                            )
                    pend_pv = _pv
                pend_pv()
                for h, out_ps in ((h_e, out_e), (h_o, out_o)):
                    ost = ostage.tile([65, QC], F32, tag="ost")
                    nc.vector.tensor_copy(ost, out_ps)
                    nc.sync.dma_start(
                        out=outT[h * 65 : (h + 1) * 65, q0 : q0 + QC], in_=ost
                    )

            # pair 0: interleave projections with attention chunks so the
            # ScalarE (attention critical path) starts as early as possible
            for qc in range(S // QC):
                qk_proj(0, [qc])
                vw_proj(qc)
                attention(0, qc)
                if qc == 1:
                    qk_proj(1, [0, 1])
                if qc == 2:
                    qk_proj(1, [2, 3])
            for pair in range(1, HEADS_PER_CORE // 2):
                for qc in range(S // QC):
                    attention(pair, qc)
                    if pair < 3 and qc < 2:
                        qk_proj(pair + 1, [2 * qc, 2 * qc + 1])
    nc.finalize()
    return nc


def _core_inputs(q, k, v, v_mask, Wq, Wk, Wv, b, hg):
    cols = slice(hg * HG_COLS, (hg + 1) * HG_COLS)
    vb = (NEG * (1.0 - v_mask[b])).astype(np.float32)
    # causal 0/1 keep-mask for the diagonal block of S^T[k, q]: keep k <= q
    # (duplicated for the two packed heads)
    ct = (np.arange(128)[:, None] <= np.arange(128)[None, :]).astype(NP_BF16)
    ct2 = np.concatenate([ct, ct], axis=1)
    return {
        "xqT": np.ascontiguousarray(q[b].T).astype(NP_BF16),
        "xkT": np.ascontiguousarray(k[b].T).astype(NP_BF16),
        "xvT": np.ascontiguousarray(v[b].T).astype(NP_BF16),
        "wq": np.ascontiguousarray(Wq[:, cols]).astype(NP_BF16),
        "wk": np.ascontiguousarray(Wk[:, cols]).astype(NP_BF16),
        "wv": np.ascontiguousarray(Wv[:, cols]).astype(NP_BF16),
        "vbias": np.ascontiguousarray(vb.reshape(NKT, 128).T),
        "ctile": np.ascontiguousarray(ct2),
    }


def kernel(q, k, v, v_mask, q_mask, Wq, Wk, Wv):
    global LAST_RESULT, _NC_CACHE
    q = np.asarray(q, np.float32)
    k = np.asarray(k, np.float32)
    v = np.asarray(v, np.float32)
    v_mask = np.asarray(v_mask, np.float32)
    q_mask = np.asarray(q_mask, np.float32)
    Wq = np.asarray(Wq, np.float32)
    Wk = np.asarray(Wk, np.float32)
    Wv = np.asarray(Wv, np.float32)

    if _NC_CACHE is None:
        _NC_CACHE = _build_nc()
    nc = _NC_CACHE

    in_maps = [
        _core_inputs(q, k, v, v_mask, Wq, Wk, Wv, c // 2, c % 2) for c in range(8)
    ]
    _ensure_ntff_hook()
    res = run_bass_kernel_spmd(nc, in_maps, core_ids=list(range(8)))
    LAST_RESULT = res

    out = np.empty((B, S, D), np.float32)
    for c in range(8):
        b, hg = c // 2, c % 2
        o = np.asarray(res.results[c]["outT"], np.float32)  # [520, 2048]
        for h in range(HEADS_PER_CORE):
            pv = o[h * 65 : h * 65 + 64, :]  # [64, S]
            sm = o[h * 65 + 64, :]  # [S]
            sm = np.where(sm == 0.0, 1.0, sm)
            g = hg * HEADS_PER_CORE + h
            out[b, :, g * 64 : (g + 1) * 64] = (pv / sm).T
    out *= q_mask[:, :, None]

    # Degenerate rows: every causally-visible key masked. The reference's
    # additive -1e10 masks then make softmax uniform over all keys with
    # v_mask=1 (causality ignored). Patch on host; never triggers unless
    # v_mask[b, 0] == 0.
    for b in range(B):
        n_pref = int(np.argmax(v_mask[b] > 0)) if v_mask[b].max() > 0 else S
        if v_mask[b, 0] == 0 and n_pref > 0:
            vw_avg = ((v_mask[b] @ v[b]) / v_mask[b].sum()) @ Wv  # [OUT_DIM]
            out[b, :n_pref, :] = vw_avg[None, :] * q_mask[b, :n_pref, None]
    return out


# revision 11
# speedup vs baseline: 1.0142x; 1.0142x over previous
"""Masked causal multi-head attention on 8 TRN2 NeuronCores.

Problem (hardcoded shapes): B=4, S=2048, D_MODEL=1024, HEADS=16,
KEY_SIZE=SIZE_PER_HEAD=64, OUT_DIM=1024, fp32 I/O.

Sharding: pure data/tensor parallel — core c handles batch b=c//2 and
head-group hg=c%2 (8 heads). Each core's output shard [2048, 512] is
independent, so there are no collectives; the host assembles shards.

Per-core pipeline (all matmul inputs bf16; PSUM accumulation fp32):
  - head-PAIR processing: even head on PE rows 0-63, odd head on rows
    64-127 (concurrent row groups) for the score matmuls; both heads'
    scores land in ONE bf16 PSUM bank [128 keys, 2, 512 q], so a
    single ScalarE activation (FD=1024) does exp for both heads —
    ScalarE instruction count (the attention-phase critical path) is
    half of the per-head variant.
  - exp((S)/8 + key_bias) with NO max-subtraction (masked lanes
    underflow to exactly 0, matching the reference's -1e10 additive
    masks); causal masking of the diagonal block is a 0/1 multiply on
    P AFTER exp (both heads in one VectorE op), off the critical path.
  - PE stream is software-pipelined: scores(kt+1) are emitted BEFORE
    PV(kt) so the PE never sits blocked on the exp of tile kt while
    score work for kt+1 is ready.
  - projections run weight-stationary (t-outer over 2 s-chunks with 2
    live PSUM accumulators) so each LDWEIGHTS is reused; pair p+1's
    qw/kw projections are emitted early inside pair p's attention so
    the Tile scheduler can fill PE gaps.
  - vw [s, dh] (bf16) carries a fused ones-column per head (65th col)
    so the PV matmul also emits softmax denominators.
  host: divide by sums, multiply q_mask, transpose, assemble.
"""

import os
import sys

import numpy as np

for _p in ("/opt/trn_rl_repo",):
    if _p not in sys.path and os.path.isdir(_p):
        sys.path.insert(0, _p)

import ml_dtypes

import concourse.bass as bass
import concourse.mybir as mybir
import concourse.tile as tile
from concourse import bacc
from concourse.bass_utils import run_bass_kernel_spmd

B = 4
S = 2048
D = 1024
HEADS_PER_CORE = 8
DH = 64
HG_COLS = HEADS_PER_CORE * DH  # 512 output cols per core
NKT = S // 128  # 16 k-tiles
NEG = -1.0e9
QC = 512  # q-chunk width (one PSUM bank)

F32 = mybir.dt.float32
BF16 = mybir.dt.bfloat16
NP_BF16 = ml_dtypes.bfloat16

LAST_RESULT = None  # stashed BassKernelResults for test harness inspection
_NC_CACHE = None


def _ensure_ntff_hook():
    """The agent image's antenv lacks axon_hooks; synthesize it so
    run_bass_kernel_spmd(trace=True) can reach the NTFF profiler."""
    try:
        from antenv.axon_hooks import get_axon_ntff_profile_hook  # noqa: F401

        return
    except ImportError:
        pass
    import types

    try:
        import antenv
        from trn_agent_boot.trn_boot import _ntff_profile_via_ctypes
    except ImportError:
        return
    mod = types.ModuleType("antenv.axon_hooks")
    _hook = [None]
    try:
        _hook[0] = _ntff_profile_via_ctypes("/opt/axon/libaxon_pjrt.so")
    except OSError:
        pass
    mod.set_axon_ntff_profile_hook = lambda h: _hook.__setitem__(0, h)
    mod.get_axon_ntff_profile_hook = lambda: _hook[0]
    sys.modules["antenv.axon_hooks"] = mod
    antenv.axon_hooks = mod


def _build_nc() -> bass.Bass:
    nc = bacc.Bacc()

    xqT = nc.declare_dram_parameter("xqT", [D, S], BF16, isOutput=False)[:]
    xkT = nc.declare_dram_parameter("xkT", [D, S], BF16, isOutput=False)[:]
    xvT = nc.declare_dram_parameter("xvT", [D, S], BF16, isOutput=False)[:]
    wq = nc.declare_dram_parameter("wq", [D, HG_COLS], BF16, isOutput=False)[:]
    wk = nc.declare_dram_parameter("wk", [D, HG_COLS], BF16, isOutput=False)[:]
    wv = nc.declare_dram_parameter("wv", [D, HG_COLS], BF16, isOutput=False)[:]
    vbias = nc.declare_dram_parameter("vbias", [128, NKT], F32, isOutput=False)[:]
    ctile = nc.declare_dram_parameter("ctile", [128, 2 * 128], BF16, isOutput=False)[:]
    outT = nc.declare_dram_parameter(
        "outT", [HEADS_PER_CORE * 65, S], F32, isOutput=True
    )[:]

    with tile.TileContext(nc) as tc:
        with (
            tc.tile_pool(name="consts", bufs=1) as consts,
            tc.tile_pool(name="xqk", bufs=1) as xqk,
            tc.tile_pool(name="wpool", bufs=1) as wpool,
            tc.tile_pool(name="qk_sb", bufs=1) as qk_sb,
            tc.tile_pool(name="vw_pool", bufs=1) as vw_pool,
            tc.tile_pool(name="xvpool", bufs=2) as xvpool,
            tc.tile_pool(name="ppool", bufs=6) as ppool,
            tc.tile_pool(name="ostage", bufs=4) as ostage,
            tc.tile_pool(name="proj_ps", bufs=2, space="PSUM") as proj_ps,
            tc.tile_pool(name="att_s", bufs=2, space="PSUM") as spool,
            tc.tile_pool(name="att_oe", bufs=1, space="PSUM") as opool_e,
            tc.tile_pool(name="att_oo", bufs=1, space="PSUM") as opool_o,
        ):
            # PE warmup: dependency-free dummy matmuls run during the DMA
            # lead-in so the HAM clock gate opens (K=8/8) before the first
            # real matmul issues
            junk = consts.tile([128, 512], BF16, tag="junk")
            nc.vector.memset(junk, 0.0)
            for _ in range(12):
                wps = proj_ps.tile([128, 512], F32, tag="pp")
                nc.tensor.matmul(
                    wps, junk[:, 0:128], junk, start=True, stop=True
                )

            # DMA strictly in need-order: the first exp gates on
            # wq/wk[dt=0] (256 KB each) + xq0/xk0 + vbias only, so those
            # come first; everything else streams in behind
            wq_sb = wpool.tile([128, 8, HG_COLS], BF16, tag="wq")
            wk_sb = wpool.tile([128, 8, HG_COLS], BF16, tag="wk")
            wq_r = wq.rearrange("(t p) n -> p t n", p=128)
            wk_r = wk.rearrange("(t p) n -> p t n", p=128)

            def dma_w_slice(dt):
                for w_sb, src in ((wq_sb, wq_r), (wk_sb, wk_r)):
                    nc.sync.dma_start(
                        out=w_sb[:, :, dt * 128 : (dt + 1) * 128],
                        in_=src[:, :, dt * 128 : (dt + 1) * 128],
                    )

            xqT_r = xqT.rearrange("(t p) s -> p t s", p=128)
            xkT_r = xkT.rearrange("(t p) s -> p t s", p=128)
            xvT_r = xvT.rearrange("(t p) s -> p t s", p=128)
            xq_cs, xk_cs, xv_ts = [], [], []

            def dma_x_chunk(sc, with_v=True, per_t=False):
                srcs = [(xqT_r, xq_cs, "xq", xqk), (xkT_r, xk_cs, "xk", xqk)]
                if with_v:
                    srcs.append((xvT_r, xv_ts, "xv", xvpool))
                for src, lst, tg, pool in srcs:
                    t_ = pool.tile([128, 8, 512], BF16, tag=f"{tg}{sc}" if pool is xqk else tg)
                    if per_t:
                        # t-granular sub-DMAs: the first projection chain
                        # starts after 128 KB instead of 1 MB
                        for t in range(8):
                            nc.sync.dma_start(
                                out=t_[:, t, :],
                                in_=src[:, t, sc * 512 : (sc + 1) * 512],
                            )
                    else:
                        nc.sync.dma_start(
                            out=t_, in_=src[:, :, sc * 512 : (sc + 1) * 512]
                        )
                    lst.append(t_)

            dma_w_slice(0)
            dma_x_chunk(0, with_v=False, per_t=True)
            vbias_sb = consts.tile([128, NKT], F32)
            nc.sync.dma_start(out=vbias_sb, in_=vbias)
            ctile_sb = consts.tile([128, 2, 128], BF16)
            nc.sync.dma_start(out=ctile_sb, in_=ctile.rearrange("p (h q) -> p h q", h=2))
            wv_r = wv.rearrange("(t p) n -> t p n", p=128)
            wv_ts = []
            for t in range(8):
                wt = wpool.tile([128, HG_COLS], BF16, tag=f"wv{t}")
                nc.sync.dma_start(out=wt, in_=wv_r[t])
                wv_ts.append(wt)
            xv0 = xvpool.tile([128, 8, 512], BF16, tag="xv", name="xv0")
            nc.sync.dma_start(out=xv0, in_=xvT_r[:, :, 0:512])
            xv_ts.append(xv0)
            for sc in range(1, S // 512):
                dma_x_chunk(sc)
            for dt in range(1, 4):
                dma_w_slice(dt)

            vw_sb = vw_pool.tile([128, NKT, HEADS_PER_CORE, 65], BF16)
            nc.vector.memset(vw_sb[:, :, :, 64:65], 1.0)

            # projected tensors
            qwT_sb = qk_sb.tile([128, 4, S], BF16)  # [dh%128, dh//128, s]
            kwT_sb = qk_sb.tile([128, 4, S], BF16)

            def vw_proj(sc):
                # project vw for k-tiles 4*sc .. 4*sc+3
                for st2 in range(4):
                    kt = sc * 4 + st2
                    ps = proj_ps.tile([128, HG_COLS], F32, tag="pp")
                    for t in range(8):
                        nc.tensor.matmul(
                            ps,
                            xv_ts[sc][:, t, st2 * 128 : (st2 + 1) * 128],
                            wv_ts[t],
                            start=(t == 0),
                            stop=(t == 7),
                        )
                    nc.vector.tensor_copy(
                        vw_sb[:, kt, :, 0:64],
                        ps.rearrange("p (h d) -> p h d", h=HEADS_PER_CORE),
                    )

            def qk_proj(dt, scs):
                # weight-stationary: t-outer over the given s-chunks with
                # one live PSUM accumulator per chunk, so each LDWEIGHTS
                # (the W tile) is reused len(scs) times
                for w_sb, x_cs, dst in (
                    (wq_sb, xq_cs, qwT_sb),
                    (wk_sb, xk_cs, kwT_sb),
                ):
                    pss = [
                        proj_ps.tile([128, 512], F32, tag="pp", name=f"pp{i}")
                        for i in range(len(scs))
                    ]
                    for t in range(8):
                        for ps, sc in zip(pss, scs):
                            nc.tensor.matmul(
                                ps,
                                w_sb[:, t, dt * 128 : (dt + 1) * 128],
                                x_cs[sc][:, t, :],
                                start=(t == 0),
                                stop=(t == 7),
                            )
                    for ps, sc in zip(pss, scs):
                        nc.vector.tensor_copy(
                            dst[:, dt, sc * 512 : (sc + 1) * 512], ps
                        )

            def attention(pair, qc):
                dt = pair
                h_e, h_o = 2 * pair, 2 * pair + 1
                q0 = qc * QC
                out_e = opool_e.tile([65, QC], F32, tag="oute")
                out_o = opool_o.tile([65, QC], F32, tag="outo")
                last_kt = (q0 + QC) // 128 - 1
                pend_pv = None
                # kt processed in groups of 2: the 4 score matmuls chain
                # back-to-back (consecutive LDWEIGHTS overlap in-flight
                # matmuls on the other row group), then 2 exps, then the
                # PREVIOUS group's 4 PV matmuls (software pipelining: the
                # PE always has ready score work while exp finishes)
                for kj in range(0, last_kt + 1, 2):
                    kts = [kt for kt in (kj, kj + 1) if kt <= last_kt]
                    p_ts = {}
                    for kt in kts:
                        q_off = max(0, 128 * kt - q0)
                        s_ps = spool.tile([128, 2 * QC], F32, tag="s")
                        for hi, poff in ((0, 0), (1, 64)):
                            nc.tensor.matmul(
                                s_ps[:, hi * QC + q_off : (hi + 1) * QC],
                                kwT_sb[
                                    poff : poff + 64, dt, kt * 128 : (kt + 1) * 128
                                ],
                                qwT_sb[poff : poff + 64, dt, q0 + q_off : q0 + QC],
                                start=True,
                                stop=True,
                            )
                        p_ts[kt] = (s_ps, q_off)
                    for kt in kts:
                        s_ps, q_off = p_ts[kt]
                        # ONE exp for both heads (FD up to 1024; flat 2D
                        # AP in the common q_off=0 case)
                        p_t = ppool.tile([128, 2 * QC], BF16, tag="p")
                        if q_off == 0:
                            act_dst, act_src = p_t, s_ps
                        else:
                            act_dst = p_t.rearrange("p (h q) -> p h q", h=2)[
                                :, :, q_off:QC
                            ]
                            act_src = s_ps.rearrange("p (h q) -> p h q", h=2)[
                                :, :, q_off:QC
                            ]
                        nc.scalar.activation(
                            act_dst,
                            act_src,
                            mybir.ActivationFunctionType.Exp,
                            bias=vbias_sb[:, kt : kt + 1],
                            scale=0.125,
                        )
                        if 128 * kt >= q0:
                            # causal mask of the diagonal block as 0/1
                            # multiply AFTER exp, both heads in one op
                            p_d = p_t.rearrange("p (h q) -> p h q", h=2)[
                                :, :, q_off : q_off + 128
                            ]
                            nc.vector.tensor_mul(p_d, p_d, ctile_sb)
                        p_ts[kt] = (p_t, q_off)
                    if pend_pv is not None:
                        pend_pv()
                    def _pv(kts=kts, p_ts=p_ts):
                        for kt in kts:
                            p_t, q_off = p_ts[kt]
                            for h, out_ps, hi in ((h_e, out_e, 0), (h_o, out_o, 1)):
                                nc.tensor.matmul(
                                    out_ps[:, q_off:QC],
                                    vw_sb[:, kt, h, :],
                                    p_t[:, hi * QC + q_off : (hi + 1) * QC],
                                    start=(kt == 0),
                                    stop=(kt == last_kt),
                                )
                    pend_pv = _pv
                pend_pv()
                for h, out_ps in ((h_e, out_e), (h_o, out_o)):
                    ost = ostage.tile([65, QC], F32, tag="ost")
                    nc.vector.tensor_copy(ost, out_ps)
                    nc.sync.dma_start(
                        out=outT[h * 65 : (h + 1) * 65, q0 : q0 + QC], in_=ost
                    )

            # pair 0: interleave projections with attention chunks so the
            # ScalarE (attention critical path) starts as early as possible
            for qc in range(S // QC):
                qk_proj(0, [qc])
                vw_proj(qc)
                attention(0, qc)
                if qc == 1:
                    qk_proj(1, [0, 1])
                if qc == 2:
                    qk_proj(1, [2, 3])
            for pair in range(1, HEADS_PER_CORE // 2):
                for qc in range(S // QC):
                    attention(pair, qc)
                    if pair < 3 and qc < 2:
                        qk_proj(pair + 1, [2 * qc, 2 * qc + 1])
    nc.finalize()
    return nc


def _core_inputs(q, k, v, v_mask, Wq, Wk, Wv, b, hg):
    cols = slice(hg * HG_COLS, (hg + 1) * HG_COLS)
    vb = (NEG * (1.0 - v_mask[b])).astype(np.float32)
    # causal 0/1 keep-mask for the diagonal block of S^T[k, q]: keep k <= q
    # (duplicated for the two packed heads)
    ct = (np.arange(128)[:, None] <= np.arange(128)[None, :]).astype(NP_BF16)
    ct2 = np.concatenate([ct, ct], axis=1)
    return {
        "xqT": np.ascontiguousarray(q[b].T).astype(NP_BF16),
        "xkT": np.ascontiguousarray(k[b].T).astype(NP_BF16),
        "xvT": np.ascontiguousarray(v[b].T).astype(NP_BF16),
        "wq": np.ascontiguousarray(Wq[:, cols]).astype(NP_BF16),
        "wk": np.ascontiguousarray(Wk[:, cols]).astype(NP_BF16),
        "wv": np.ascontiguousarray(Wv[:, cols]).astype(NP_BF16),
        "vbias": np.ascontiguousarray(vb.reshape(NKT, 128).T),
        "ctile": np.ascontiguousarray(ct2),
    }


def kernel(q, k, v, v_mask, q_mask, Wq, Wk, Wv):
    global LAST_RESULT, _NC_CACHE
    q = np.asarray(q, np.float32)
    k = np.asarray(k, np.float32)
    v = np.asarray(v, np.float32)
    v_mask = np.asarray(v_mask, np.float32)
    q_mask = np.asarray(q_mask, np.float32)
    Wq = np.asarray(Wq, np.float32)
    Wk = np.asarray(Wk, np.float32)
    Wv = np.asarray(Wv, np.float32)

    if _NC_CACHE is None:
        _NC_CACHE = _build_nc()
    nc = _NC_CACHE

    in_maps = [
        _core_inputs(q, k, v, v_mask, Wq, Wk, Wv, c // 2, c % 2) for c in range(8)
    ]
    _ensure_ntff_hook()
    res = run_bass_kernel_spmd(nc, in_maps, core_ids=list(range(8)))
    LAST_RESULT = res

    out = np.empty((B, S, D), np.float32)
    for c in range(8):
        b, hg = c // 2, c % 2
        o = np.asarray(res.results[c]["outT"], np.float32)  # [520, 2048]
        for h in range(HEADS_PER_CORE):
            pv = o[h * 65 : h * 65 + 64, :]  # [64, S]
            sm = o[h * 65 + 64, :]  # [S]
            sm = np.where(sm == 0.0, 1.0, sm)
            g = hg * HEADS_PER_CORE + h
            out[b, :, g * 64 : (g + 1) * 64] = (pv / sm).T
    out *= q_mask[:, :, None]

    # Degenerate rows: every causally-visible key masked. The reference's
    # additive -1e10 masks then make softmax uniform over all keys with
    # v_mask=1 (causality ignored). Patch on host; never triggers unless
    # v_mask[b, 0] == 0.
    for b in range(B):
        n_pref = int(np.argmax(v_mask[b] > 0)) if v_mask[b].max() > 0 else S
        if v_mask[b, 0] == 0 and n_pref > 0:
            vw_avg = ((v_mask[b] @ v[b]) / v_mask[b].sum()) @ Wv  # [OUT_DIM]
            out[b, :n_pref, :] = vw_avg[None, :] * q_mask[b, :n_pref, None]
    return out


# revision 13
# speedup vs baseline: 1.0247x; 1.0104x over previous
"""Masked causal multi-head attention on 8 TRN2 NeuronCores.

Problem (hardcoded shapes): B=4, S=2048, D_MODEL=1024, HEADS=16,
KEY_SIZE=SIZE_PER_HEAD=64, OUT_DIM=1024, fp32 I/O.

Sharding: pure data/tensor parallel — core c handles batch b=c//2 and
head-group hg=c%2 (8 heads). Each core's output shard [2048, 512] is
independent, so there are no collectives; the host assembles shards.

Per-core pipeline (all matmul inputs bf16; PSUM accumulation fp32):
  - head-PAIR processing: even head on PE rows 0-63, odd head on rows
    64-127 (concurrent row groups) for the score matmuls; both heads'
    scores land in ONE bf16 PSUM bank [128 keys, 2, 512 q], so a
    single ScalarE activation (FD=1024) does exp for both heads —
    ScalarE instruction count (the attention-phase critical path) is
    half of the per-head variant.
  - exp((S)/8 + key_bias) with NO max-subtraction (masked lanes
    underflow to exactly 0, matching the reference's -1e10 additive
    masks); causal masking of the diagonal block is a 0/1 multiply on
    P AFTER exp (both heads in one VectorE op), off the critical path.
  - PE stream is software-pipelined: scores(kt+1) are emitted BEFORE
    PV(kt) so the PE never sits blocked on the exp of tile kt while
    score work for kt+1 is ready.
  - projections run weight-stationary (t-outer over 2 s-chunks with 2
    live PSUM accumulators) so each LDWEIGHTS is reused; pair p+1's
    qw/kw projections are emitted early inside pair p's attention so
    the Tile scheduler can fill PE gaps.
  - vw [s, dh] (bf16) carries a fused ones-column per head (65th col)
    so the PV matmul also emits softmax denominators.
  host: divide by sums, multiply q_mask, transpose, assemble.
"""

import os
import sys

import numpy as np

for _p in ("/opt/trn_rl_repo",):
    if _p not in sys.path and os.path.isdir(_p):
        sys.path.insert(0, _p)

import ml_dtypes

import concourse.bass as bass
import concourse.mybir as mybir
import concourse.tile as tile
from concourse import bacc
from concourse.bass_utils import run_bass_kernel_spmd

B = 4
S = 2048
D = 1024
HEADS_PER_CORE = 8
DH = 64
HG_COLS = HEADS_PER_CORE * DH  # 512 output cols per core
NKT = S // 128  # 16 k-tiles
NEG = -1.0e9
QC = 512  # q-chunk width (one PSUM bank)

F32 = mybir.dt.float32
BF16 = mybir.dt.bfloat16
NP_BF16 = ml_dtypes.bfloat16

LAST_RESULT = None  # stashed BassKernelResults for test harness inspection
_NC_CACHE = None


def _ensure_ntff_hook():
    """The agent image's antenv lacks axon_hooks; synthesize it so
    run_bass_kernel_spmd(trace=True) can reach the NTFF profiler."""
    try:
        from antenv.axon_hooks import get_axon_ntff_profile_hook  # noqa: F401

        return
    except ImportError:
        pass
    import types

    try:
        import antenv
        from trn_agent_boot.trn_boot import _ntff_profile_via_ctypes
    except ImportError:
        return
    mod = types.ModuleType("antenv.axon_hooks")
    _hook = [None]
    try:
        _hook[0] = _ntff_profile_via_ctypes("/opt/axon/libaxon_pjrt.so")
    except OSError:
        pass
    mod.set_axon_ntff_profile_hook = lambda h: _hook.__setitem__(0, h)
    mod.get_axon_ntff_profile_hook = lambda: _hook[0]
    sys.modules["antenv.axon_hooks"] = mod
    antenv.axon_hooks = mod


def _build_nc() -> bass.Bass:
    nc = bacc.Bacc()

    xqT = nc.declare_dram_parameter("xqT", [D, S], BF16, isOutput=False)[:]
    xkT = nc.declare_dram_parameter("xkT", [D, S], BF16, isOutput=False)[:]
    xvT = nc.declare_dram_parameter("xvT", [D, S], BF16, isOutput=False)[:]
    wq = nc.declare_dram_parameter("wq", [D, HG_COLS], BF16, isOutput=False)[:]
    wk = nc.declare_dram_parameter("wk", [D, HG_COLS], BF16, isOutput=False)[:]
    wv = nc.declare_dram_parameter("wv", [D, HG_COLS], BF16, isOutput=False)[:]
    vbias = nc.declare_dram_parameter("vbias", [128, NKT], F32, isOutput=False)[:]
    ctile = nc.declare_dram_parameter("ctile", [128, 2 * 128], BF16, isOutput=False)[:]
    outT = nc.declare_dram_parameter(
        "outT", [HEADS_PER_CORE * 65, S], F32, isOutput=True
    )[:]

    with tile.TileContext(nc) as tc:
        with (
            tc.tile_pool(name="consts", bufs=1) as consts,
            tc.tile_pool(name="xqk", bufs=1) as xqk,
            tc.tile_pool(name="wpool", bufs=1) as wpool,
            tc.tile_pool(name="qk_sb", bufs=1) as qk_sb,
            tc.tile_pool(name="vw_pool", bufs=1) as vw_pool,
            tc.tile_pool(name="xvpool", bufs=2) as xvpool,
            tc.tile_pool(name="ppool", bufs=6) as ppool,
            tc.tile_pool(name="ostage", bufs=4) as ostage,
            tc.tile_pool(name="proj_ps", bufs=2, space="PSUM") as proj_ps,
            tc.tile_pool(name="att_s", bufs=2, space="PSUM") as spool,
            tc.tile_pool(name="att_oe", bufs=1, space="PSUM") as opool_e,
            tc.tile_pool(name="att_oo", bufs=1, space="PSUM") as opool_o,
        ):
            # PE warmup: dependency-free dummy matmuls run during the DMA
            # lead-in so the HAM clock gate opens (K=8/8) before the first
            # real matmul issues
            junk = consts.tile([128, 512], BF16, tag="junk")
            nc.vector.memset(junk, 0.0)
            for _ in range(12):
                wps = proj_ps.tile([128, 512], F32, tag="pp")
                nc.tensor.matmul(
                    wps, junk[:, 0:128], junk, start=True, stop=True
                )

            # DMA strictly in need-order: the first exp gates on
            # wq/wk[dt=0] (256 KB each) + xq0/xk0 + vbias only, so those
            # come first; everything else streams in behind
            wq_sb = wpool.tile([128, 8, HG_COLS], BF16, tag="wq")
            wk_sb = wpool.tile([128, 8, HG_COLS], BF16, tag="wk")
            wq_r = wq.rearrange("(t p) n -> p t n", p=128)
            wk_r = wk.rearrange("(t p) n -> p t n", p=128)

            def dma_w_slice(dt):
                for w_sb, src in ((wq_sb, wq_r), (wk_sb, wk_r)):
                    nc.sync.dma_start(
                        out=w_sb[:, :, dt * 128 : (dt + 1) * 128],
                        in_=src[:, :, dt * 128 : (dt + 1) * 128],
                    )

            xqT_r = xqT.rearrange("(t p) s -> p t s", p=128)
            xkT_r = xkT.rearrange("(t p) s -> p t s", p=128)
            xvT_r = xvT.rearrange("(t p) s -> p t s", p=128)
            xq_cs, xk_cs, xv_ts = [], [], []

            def dma_x_chunk(sc, with_v=True, per_t=False):
                srcs = []
                if with_v:
                    srcs.append((xvT_r, xv_ts, "xv", xvpool))
                srcs += [(xqT_r, xq_cs, "xq", xqk), (xkT_r, xk_cs, "xk", xqk)]
                for src, lst, tg, pool in srcs:
                    t_ = pool.tile([128, 8, 512], BF16, tag=f"{tg}{sc}" if pool is xqk else tg)
                    if per_t:
                        # t-granular sub-DMAs: the first projection chain
                        # starts after 128 KB instead of 1 MB
                        for t in range(8):
                            nc.sync.dma_start(
                                out=t_[:, t, :],
                                in_=src[:, t, sc * 512 : (sc + 1) * 512],
                            )
                    else:
                        nc.sync.dma_start(
                            out=t_, in_=src[:, :, sc * 512 : (sc + 1) * 512]
                        )
                    lst.append(t_)

            dma_w_slice(0)
            dma_x_chunk(0, with_v=False, per_t=True)
            vbias_sb = consts.tile([128, NKT], F32)
            nc.sync.dma_start(out=vbias_sb, in_=vbias)
            ctile_sb = consts.tile([128, 2, 128], BF16)
            nc.sync.dma_start(out=ctile_sb, in_=ctile.rearrange("p (h q) -> p h q", h=2))
            wv_r = wv.rearrange("(t p) n -> t p n", p=128)
            wv_ts = []
            for t in range(8):
                wt = wpool.tile([128, HG_COLS], BF16, tag=f"wv{t}")
                nc.sync.dma_start(out=wt, in_=wv_r[t])
                wv_ts.append(wt)
            xv0 = xvpool.tile([128, 8, 512], BF16, tag="xv", name="xv0")
            nc.sync.dma_start(out=xv0, in_=xvT_r[:, :, 0:512])
            xv_ts.append(xv0)
            dma_x_chunk(1)
            # pair-1 projections (emitted inside pair-0 attention) need
            # the dt=1 weight slices well before the later x chunks
            dma_w_slice(1)
            dma_x_chunk(2)
            dma_w_slice(2)
            dma_x_chunk(3)
            dma_w_slice(3)

            vw_sb = vw_pool.tile([128, NKT, HEADS_PER_CORE, 65], BF16)
            nc.vector.memset(vw_sb[:, :, :, 64:65], 1.0)

            # projected tensors
            qwT_sb = qk_sb.tile([128, 4, S], BF16)  # [dh%128, dh//128, s]
            kwT_sb = qk_sb.tile([128, 4, S], BF16)

            def vw_proj(sc):
                # project vw for k-tiles 4*sc .. 4*sc+3
                for st2 in range(4):
                    kt = sc * 4 + st2
                    ps = proj_ps.tile([128, HG_COLS], F32, tag="pp")
                    for t in range(8):
                        nc.tensor.matmul(
                            ps,
                            xv_ts[sc][:, t, st2 * 128 : (st2 + 1) * 128],
                            wv_ts[t],
                            start=(t == 0),
                            stop=(t == 7),
                        )
                    nc.vector.tensor_copy(
                        vw_sb[:, kt, :, 0:64],
                        ps.rearrange("p (h d) -> p h d", h=HEADS_PER_CORE),
                    )

            def qk_proj(dt, scs):
                # weight-stationary: t-outer over the given s-chunks with
                # one live PSUM accumulator per chunk, so each LDWEIGHTS
                # (the W tile) is reused len(scs) times
                for w_sb, x_cs, dst in (
                    (wq_sb, xq_cs, qwT_sb),
                    (wk_sb, xk_cs, kwT_sb),
                ):
                    pss = [
                        proj_ps.tile([128, 512], F32, tag="pp", name=f"pp{i}")
                        for i in range(len(scs))
                    ]
                    for t in range(8):
                        for ps, sc in zip(pss, scs):
                            nc.tensor.matmul(
                                ps,
                                w_sb[:, t, dt * 128 : (dt + 1) * 128],
                                x_cs[sc][:, t, :],
                                start=(t == 0),
                                stop=(t == 7),
                            )
                    for ps, sc in zip(pss, scs):
                        nc.vector.tensor_copy(
                            dst[:, dt, sc * 512 : (sc + 1) * 512], ps
                        )

            def attention(pair, qc):
                dt = pair
                h_e, h_o = 2 * pair, 2 * pair + 1
                q0 = qc * QC
                out_e = opool_e.tile([65, QC], F32, tag="oute")
                out_o = opool_o.tile([65, QC], F32, tag="outo")
                last_kt = (q0 + QC) // 128 - 1
                pend_pv = None
                # kt processed in groups of 2: the 4 score matmuls chain
                # back-to-back (consecutive LDWEIGHTS overlap in-flight
                # matmuls on the other row group), then 2 exps, then the
                # PREVIOUS group's 4 PV matmuls (software pipelining: the
                # PE always has ready score work while exp finishes)
                for kj in range(0, last_kt + 1, 2):
                    kts = [kt for kt in (kj, kj + 1) if kt <= last_kt]
                    p_ts = {}
                    for kt in kts:
                        q_off = max(0, 128 * kt - q0)
                        s_ps = spool.tile([128, 2 * QC], F32, tag="s")
                        for hi, poff in ((0, 0), (1, 64)):
                            nc.tensor.matmul(
                                s_ps[:, hi * QC + q_off : (hi + 1) * QC],
                                kwT_sb[
                                    poff : poff + 64, dt, kt * 128 : (kt + 1) * 128
                                ],
                                qwT_sb[poff : poff + 64, dt, q0 + q_off : q0 + QC],
                                start=True,
                                stop=True,
                            )
                        p_ts[kt] = (s_ps, q_off)
                    for kt in kts:
                        s_ps, q_off = p_ts[kt]
                        # ONE exp for both heads (FD up to 1024; flat 2D
                        # AP in the common q_off=0 case)
                        p_t = ppool.tile([128, 2 * QC], BF16, tag="p")
                        if q_off == 0:
                            act_dst, act_src = p_t, s_ps
                        else:
                            act_dst = p_t.rearrange("p (h q) -> p h q", h=2)[
                                :, :, q_off:QC
                            ]
                            act_src = s_ps.rearrange("p (h q) -> p h q", h=2)[
                                :, :, q_off:QC
                            ]
                        nc.scalar.activation(
                            act_dst,
                            act_src,
                            mybir.ActivationFunctionType.Exp,
                            bias=vbias_sb[:, kt : kt + 1],
                            scale=0.125,
                        )
                        if 128 * kt >= q0:
                            # causal mask of the diagonal block as 0/1
                            # multiply AFTER exp, both heads in one op
                            p_d = p_t.rearrange("p (h q) -> p h q", h=2)[
                                :, :, q_off : q_off + 128
                            ]
                            nc.vector.tensor_mul(p_d, p_d, ctile_sb)
                        p_ts[kt] = (p_t, q_off)
                    if pend_pv is not None:
                        pend_pv()
                    def _pv(kts=kts, p_ts=p_ts):
                        for kt in kts:
                            p_t, q_off = p_ts[kt]
                            for h, out_ps, hi in ((h_e, out_e, 0), (h_o, out_o, 1)):
                                nc.tensor.matmul(
                                    out_ps[:, q_off:QC],
                                    vw_sb[:, kt, h, :],
                                    p_t[:, hi * QC + q_off : (hi + 1) * QC],
                                    start=(kt == 0),
                                    stop=(kt == last_kt),
                                )
                    pend_pv = _pv
                pend_pv()
                for h, out_ps in ((h_e, out_e), (h_o, out_o)):
                    ost = ostage.tile([65, QC], F32, tag="ost")
                    nc.vector.tensor_copy(ost, out_ps)
                    nc.sync.dma_start(
                        out=outT[h * 65 : (h + 1) * 65, q0 : q0 + QC], in_=ost
                    )

            # pair 0: interleave projections with attention chunks so the
            # ScalarE (attention critical path) starts as early as possible
            for qc in range(S // QC):
                qk_proj(0, [qc])
                vw_proj(qc)
                attention(0, qc)
                if qc == 1:
                    qk_proj(1, [0, 1])
                if qc == 2:
                    qk_proj(1, [2, 3])
            for pair in range(1, HEADS_PER_CORE // 2):
                for qc in range(S // QC):
                    attention(pair, qc)
                    if pair < 3 and qc < 2:
                        qk_proj(pair + 1, [2 * qc, 2 * qc + 1])
    nc.finalize()
    return nc


def _core_inputs(q, k, v, v_mask, Wq, Wk, Wv, b, hg):
    cols = slice(hg * HG_COLS, (hg + 1) * HG_COLS)
    vb = (NEG * (1.0 - v_mask[b])).astype(np.float32)
    # causal 0/1 keep-mask for the diagonal block of S^T[k, q]: keep k <= q
    # (duplicated for the two packed heads)
    ct = (np.arange(128)[:, None] <= np.arange(128)[None, :]).astype(NP_BF16)
    ct2 = np.concatenate([ct, ct], axis=1)
    return {
        "xqT": np.ascontiguousarray(q[b].T).astype(NP_BF16),
        "xkT": np.ascontiguousarray(k[b].T).astype(NP_BF16),
        "xvT": np.ascontiguousarray(v[b].T).astype(NP_BF16),
        "wq": np.ascontiguousarray(Wq[:, cols]).astype(NP_BF16),
        "wk": np.ascontiguousarray(Wk[:, cols]).astype(NP_BF16),
        "wv": np.ascontiguousarray(Wv[:, cols]).astype(NP_BF16),
        "vbias": np.ascontiguousarray(vb.reshape(NKT, 128).T),
        "ctile": np.ascontiguousarray(ct2),
    }


def kernel(q, k, v, v_mask, q_mask, Wq, Wk, Wv):
    global LAST_RESULT, _NC_CACHE
    q = np.asarray(q, np.float32)
    k = np.asarray(k, np.float32)
    v = np.asarray(v, np.float32)
    v_mask = np.asarray(v_mask, np.float32)
    q_mask = np.asarray(q_mask, np.float32)
    Wq = np.asarray(Wq, np.float32)
    Wk = np.asarray(Wk, np.float32)
    Wv = np.asarray(Wv, np.float32)

    if _NC_CACHE is None:
        _NC_CACHE = _build_nc()
    nc = _NC_CACHE

    in_maps = [
        _core_inputs(q, k, v, v_mask, Wq, Wk, Wv, c // 2, c % 2) for c in range(8)
    ]
    _ensure_ntff_hook()
    res = run_bass_kernel_spmd(nc, in_maps, core_ids=list(range(8)))
    LAST_RESULT = res

    out = np.empty((B, S, D), np.float32)
    for c in range(8):
        b, hg = c // 2, c % 2
        o = np.asarray(res.results[c]["outT"], np.float32)  # [520, 2048]
        for h in range(HEADS_PER_CORE):
            pv = o[h * 65 : h * 65 + 64, :]  # [64, S]
            sm = o[h * 65 + 64, :]  # [S]
            sm = np.where(sm == 0.0, 1.0, sm)
            g = hg * HEADS_PER_CORE + h
            out[b, :, g * 64 : (g + 1) * 64] = (pv / sm).T
    out *= q_mask[:, :, None]

    # Degenerate rows: every causally-visible key masked. The reference's
    # additive -1e10 masks then make softmax uniform over all keys with
    # v_mask=1 (causality ignored). Patch on host; never triggers unless
    # v_mask[b, 0] == 0.
    for b in range(B):
        n_pref = int(np.argmax(v_mask[b] > 0)) if v_mask[b].max() > 0 else S
        if v_mask[b, 0] == 0 and n_pref > 0:
            vw_avg = ((v_mask[b] @ v[b]) / v_mask[b].sum()) @ Wv  # [OUT_DIM]
            out[b, :n_pref, :] = vw_avg[None, :] * q_mask[b, :n_pref, None]
    return out
